# revision 6
# baseline (speedup 1.0000x reference)
"""AdaModConv1D on 8 TRN2 NeuronCores — pure data parallel (1 sample/core).

Math: s = softplus(ltnt @ Wd + bd) + 1          [B, C]
      d = rsqrt(einsum('kcf,bc->bf', K^2, s^2) + eps)
      y = conv1d(x * s, K, SAME) * d

Per-core trick: each core owns ONE sample, so the input modulation (x*s) and
output demodulation (y*d) fold into the conv weights:
      w''[k,c,f] = K[k,c,f] * s[c] * d[f];   y = conv1d(x, w'')
making the device inner loop a pure 3-tap conv1d = 3 accumulating matmuls
per output tile on the PE, with a tiny on-device prologue computing w''.

Layout: host pre-transposes each sample to channels-first bf16
[128 = (half, c), 32770] with one halo column on each side (half = L split in
two so both partition halves are used; the two 64x64 conv quadrants run at
PE tile_position (0,0) and (64,64) concurrently). Output comes back as
y^T [128 = (half, f), 32768] bf16 and is re-transposed on host.
"""

import os
import sys

sys.path.insert(0, "/opt/trn_rl_repo")

import numpy as np
import ml_dtypes

BF16 = ml_dtypes.bfloat16

B, L, C = 8, 65536, 64
F, KW, DL = 64, 3, 256
EPS = 1e-8
H = L // 2          # 32768, half length per partition group
NCHUNK = 8          # DMA chunks per direction
CHW = H // NCHUNK   # 4096 columns per chunk
NGRP = H // 512     # 64 matmul groups of 512 outputs x 2 halves

_cached = {}


def _build():
    import concourse.bass as bass
    import concourse.bacc as bacc
    import concourse.mybir as mybir
    import concourse.tile as tile

    dt = mybir.dt
    nc = bacc.Bacc("TRN2", target_bir_lowering=False, debug=False, num_devices=8)

    xin = nc.declare_dram_parameter("xin", [128, H + 2], dt.bfloat16, isOutput=False)
    lt = nc.declare_dram_parameter("lt", [DL], dt.float32, isOutput=False)
    ker = nc.declare_dram_parameter("ker", [KW, C, F], dt.float32, isOutput=False)
    wd = nc.declare_dram_parameter("wd", [DL, F], dt.float32, isOutput=False)
    bd = nc.declare_dram_parameter("bd", [F], dt.float32, isOutput=False)
    yout = nc.declare_dram_parameter("yout", [128, H], dt.bfloat16, isOutput=True)

    with tile.TileContext(nc) as tc:
        with (
            tc.tile_pool(name="xin", bufs=1) as xin_pool,
            tc.tile_pool(name="yout", bufs=1) as yout_pool,
            tc.tile_pool(name="pre", bufs=1) as pre,
            tc.tile_pool(name="pp", bufs=2, space="PSUM") as pre_psum,
            tc.tile_pool(name="cp", bufs=6, space="PSUM") as conv_psum,
        ):
            # ---- input chunk DMAs (issued first so they stream early) ----
            xc = []
            for c in range(NCHUNK):
                t = xin_pool.tile([128, CHW + 2], dt.bfloat16, tag=f"xin{c}")
                nc.sync.dma_start(out=t[:], in_=xin[:, c * CHW : c * CHW + CHW + 2])
                xc.append(t)

            # ---- prologue: small params in ----
            wd_sb = pre.tile([128, 2, 2, F], dt.float32, tag="wd")
            for a in range(2):
                src = wd[a * 128 : (a + 1) * 128, :]
                for h in range(2):
                    nc.sync.dma_start(out=wd_sb[:, a, h, :], in_=src)
            lt_sb = pre.tile([128, 2], dt.float32, tag="lt")
            lt2 = lt.rearrange("(a p) -> a p", a=2)
            for a in range(2):
                nc.sync.dma_start(out=lt_sb[:, a : a + 1], in_=lt2[a, :, None])
            bd_sb = pre.tile([128, 1], dt.float32, tag="bd")
            for h in range(2):
                nc.sync.dma_start(out=bd_sb[h * 64 : (h + 1) * 64, :], in_=bd[:, None])
            ker_sb = pre.tile([128, KW, F], dt.float32, tag="ker")
            kr = ker.rearrange("k c f -> c k f")
            for h in range(2):
                nc.sync.dma_start(out=ker_sb[h * 64 : (h + 1) * 64, :, :], in_=kr)
            ker_flat = ker_sb.rearrange("p k f -> p (k f)")

            # ---- prologue: s = softplus(ltnt @ Wd + bd) + 1, per (half, c) ----
            s_pre = pre_psum.tile([128, 1], dt.float32, tag="pp")
            for a in range(2):
                nc.tensor.matmul(
                    s_pre[:],
                    lhsT=wd_sb[:, a, :, :],
                    rhs=lt_sb[:, a : a + 1],
                    start=(a == 0),
                    stop=(a == 1),
                )
            # softplus(p) = ln(1 + exp(p)); only exp/ln/copy exist in one ACT set
            e_sb = pre.tile([128, 1], dt.float32, tag="e")
            nc.scalar.activation(
                e_sb[:], s_pre[:], mybir.ActivationFunctionType.Exp, bias=bd_sb[:]
            )
            nc.vector.tensor_scalar_add(e_sb[:], e_sb[:], 1.0)
            s_sb = pre.tile([128, 1], dt.float32, tag="s")
            nc.scalar.activation(s_sb[:], e_sb[:], mybir.ActivationFunctionType.Ln)
            nc.vector.tensor_scalar_add(s_sb[:], s_sb[:], 1.0)
            s2_sb = pre.tile([128, 1], dt.float32, tag="s2")
            nc.vector.tensor_mul(s2_sb[:], s_sb[:], s_sb[:])

            # ---- prologue: d = 1/sqrt(sum_kc K^2 s^2 + eps), as [1, F] ----
            k2_sb = pre.tile([128, KW * F], dt.float32, tag="k2")
            nc.vector.tensor_mul(k2_sb[:], ker_flat[:], ker_flat[:])
            dpre = pre_psum.tile([1, F], dt.float32, tag="pp")
            for k in range(KW):
                nc.tensor.matmul(
                    dpre[:],
                    lhsT=s2_sb[0:64, :],
                    rhs=k2_sb[0:64, k * F : (k + 1) * F],
                    start=(k == 0),
                    stop=(k == KW - 1),
                )
            # rsqrt(v) = exp(-0.5 * ln(v)) — keeps ACT funcs within one LUT set
            eps_sb = pre.tile([1, 1], dt.float32, tag="eps")
            nc.vector.memset(eps_sb[:], EPS)
            lnv = pre.tile([1, F], dt.float32, tag="lnv")
            nc.scalar.activation(
                lnv[:], dpre[:], mybir.ActivationFunctionType.Ln, bias=eps_sb[:]
            )
            d_sb = pre.tile([1, F], dt.float32, tag="d")
            nc.scalar.activation(
                d_sb[:], lnv[:], mybir.ActivationFunctionType.Exp, scale=-0.5
            )
            d3_sb = pre.tile([1, KW * F], dt.float32, tag="d3")
            for k in range(KW):
                nc.vector.tensor_copy(d3_sb[:, k * F : (k + 1) * F], d_sb[:])

            # ---- prologue: w''[(h,c),(k,f)] = K[k,c,f] * d[f] * s[c], bf16 ----
            ones = pre.tile([1, 64], dt.float32, tag="ones")
            nc.vector.memset(ones[:], 1.0)
            dmat = pre_psum.tile([128, KW * F], dt.float32, tag="pp")
            for h in range(2):
                nc.tensor.matmul(
                    dmat[h * 64 : (h + 1) * 64, :],
                    lhsT=ones[:],
                    rhs=d3_sb[:],
                    start=True,
                    stop=True,
                )
            wtmp = pre.tile([128, KW * F], dt.float32, tag="wtmp")
            nc.vector.tensor_mul(wtmp[:], ker_flat[:], dmat[:])
            wfin = pre.tile([128, KW * F], dt.bfloat16, tag="wfin")
            nc.vector.tensor_scalar_mul(wfin[:], wtmp[:], s_sb[:])

            # ---- main conv loop ----
            yc = [
                yout_pool.tile(
                    [128, CHW], dt.bfloat16, name=f"yout{c}", tag=f"yout{c}"
                )
                for c in range(NCHUNK)
            ]
            for g in range(NGRP):
                c, j = divmod(g, NGRP // NCHUNK)
                base = j * 512
                ps = conv_psum.tile([128, 512], dt.float32, tag="convps")
                for k in range(KW):
                    st, sp = (k == 0), (k == KW - 1)
                    for h in range(2):
                        nc.tensor.matmul(
                            ps[h * 64 : (h + 1) * 64, :],
                            lhsT=wfin[h * 64 : (h + 1) * 64, k * F : (k + 1) * F],
                            rhs=xc[c][h * 64 : (h + 1) * 64, base + k : base + k + 512],
                            start=st,
                            stop=sp,
                            skip_group_check=True,
                        )
                dst = yc[c][:, base : base + 512]
                if g % 2 == 0:
                    nc.vector.tensor_copy(dst, ps[:])
                else:
                    nc.scalar.copy(dst, ps[:])
                if j == NGRP // NCHUNK - 1:
                    nc.scalar.dma_start(
                        out=yout[:, c * CHW : (c + 1) * CHW], in_=yc[c][:]
                    )

    nc.compile()
    return nc


def _get_nc():
    if "nc" not in _cached:
        _cached["nc"] = _build()
    return _cached["nc"]


def kernel(data, ltnt, kernel, Wd, bd):
    from concourse import bass_utils

    nc = _get_nc()

    data = np.asarray(data, dtype=np.float32)
    ltnt = np.asarray(ltnt, dtype=np.float32)
    kf = np.ascontiguousarray(np.asarray(kernel, dtype=np.float32))
    wdf = np.ascontiguousarray(np.asarray(Wd, dtype=np.float32))
    bdf = np.ascontiguousarray(np.asarray(bd, dtype=np.float32))

    in_maps = []
    for b in range(B):
        xt = data[b].reshape(2, H, C).transpose(0, 2, 1)  # [2, C, H]
        xin = np.zeros((128, H + 2), dtype=BF16)
        xin[:, 1 : H + 1] = xt.reshape(128, H).astype(BF16)
        xin[64:128, 0] = xt[0, :, -1].astype(BF16)  # x[H-1] left halo of half 1
        xin[0:64, H + 1] = xt[1, :, 0].astype(BF16)  # x[H] right halo of half 0
        in_maps.append(
            {
                "xin": xin,
                "lt": np.ascontiguousarray(ltnt[b]),
                "ker": kf,
                "wd": wdf,
                "bd": bdf,
            }
        )

    res = bass_utils.run_bass_kernel_spmd(nc, in_maps, core_ids=list(range(B)))

    out = np.empty((B, L, C), dtype=np.float32)
    for b in range(B):
        yo = np.asarray(res.results[b]["yout"])  # [128, H] bf16
        out[b] = (
            yo.reshape(2, F, H).transpose(0, 2, 1).reshape(L, F).astype(np.float32)
        )
    return out


# revision 8
# speedup vs baseline: 1.0188x; 1.0188x over previous
"""AdaModConv1D on 8 TRN2 NeuronCores — pure data parallel (1 sample/core).

Math: s = softplus(ltnt @ Wd + bd) + 1          [B, C]
      d = rsqrt(einsum('kcf,bc->bf', K^2, s^2) + eps)
      y = conv1d(x * s, K, SAME) * d

Per-core trick: each core owns ONE sample, so the input modulation (x*s) and
output demodulation (y*d) fold into the conv weights:
      w''[k,c,f] = K[k,c,f] * s[c] * d[f];   y = conv1d(x, w'')
making the device inner loop a pure 3-tap conv1d = 3 accumulating matmuls
per output tile on the PE, with a tiny on-device prologue computing w''.

Layout: host pre-transposes each sample to channels-first bf16
[128 = (half, c), 32770] with one halo column on each side (half = L split in
two so both partition halves are used; the two 64x64 conv quadrants run at
PE tile_position (0,0) and (64,64) concurrently). Output comes back as
y^T [128 = (half, f), 32768] bf16 and is re-transposed on host.
"""

import os
import sys

sys.path.insert(0, "/opt/trn_rl_repo")

import numpy as np
import ml_dtypes

BF16 = ml_dtypes.bfloat16

B, L, C = 8, 65536, 64
F, KW, DL = 64, 3, 256
EPS = 1e-8
H = L // 2          # 32768, half length per partition group
NCHUNK = 8          # DMA chunks per direction
CHW = H // NCHUNK   # 4096 columns per chunk
NGRP = H // 512     # 64 matmul groups of 512 outputs x 2 halves

_cached = {}


def _build():
    import concourse.bass as bass
    import concourse.bacc as bacc
    import concourse.mybir as mybir
    import concourse.tile as tile

    dt = mybir.dt
    nc = bacc.Bacc("TRN2", target_bir_lowering=False, debug=False, num_devices=8)

    xin = nc.declare_dram_parameter("xin", [128, H + 2], dt.bfloat16, isOutput=False)
    lt = nc.declare_dram_parameter("lt", [DL], dt.float32, isOutput=False)
    ker = nc.declare_dram_parameter("ker", [KW, C, F], dt.float32, isOutput=False)
    wd = nc.declare_dram_parameter("wd", [DL, F], dt.float32, isOutput=False)
    bd = nc.declare_dram_parameter("bd", [F], dt.float32, isOutput=False)
    yout = nc.declare_dram_parameter("yout", [128, H], dt.bfloat16, isOutput=True)

    with tile.TileContext(nc) as tc:
        with (
            tc.tile_pool(name="xin", bufs=1) as xin_pool,
            tc.tile_pool(name="yout", bufs=1) as yout_pool,
            tc.tile_pool(name="pre", bufs=1) as pre,
            tc.tile_pool(name="pp", bufs=2, space="PSUM") as pre_psum,
            tc.tile_pool(name="cp", bufs=6, space="PSUM") as conv_psum,
        ):
            # ---- input chunk DMAs (issued first so they stream early) ----
            xc = []
            for c in range(NCHUNK):
                t = xin_pool.tile([128, CHW + 2], dt.bfloat16, tag=f"xin{c}")
                nc.sync.dma_start(out=t[:], in_=xin[:, c * CHW : c * CHW + CHW + 2])
                xc.append(t)

            # ---- prologue: small params in (scalar HWDGE ring — must not
            # queue behind the big input chunks on the sync ring) ----
            wd_sb = pre.tile([128, 2, 2, F], dt.float32, tag="wd")
            for a in range(2):
                src = wd[a * 128 : (a + 1) * 128, :]
                for h in range(2):
                    nc.scalar.dma_start(out=wd_sb[:, a, h, :], in_=src)
            lt_sb = pre.tile([128, 2], dt.float32, tag="lt")
            lt2 = lt.rearrange("(a p) -> a p", a=2)
            for a in range(2):
                nc.scalar.dma_start(out=lt_sb[:, a : a + 1], in_=lt2[a, :, None])
            bd_sb = pre.tile([128, 1], dt.float32, tag="bd")
            for h in range(2):
                nc.scalar.dma_start(
                    out=bd_sb[h * 64 : (h + 1) * 64, :], in_=bd[:, None]
                )
            ker_sb = pre.tile([128, KW, F], dt.float32, tag="ker")
            kr = ker.rearrange("k c f -> c k f")
            for h in range(2):
                nc.scalar.dma_start(out=ker_sb[h * 64 : (h + 1) * 64, :, :], in_=kr)
            ker_flat = ker_sb.rearrange("p k f -> p (k f)")

            # ---- PE warm-up: keep TensorE busy ~3.5us so HAM is at 2.4GHz
            # when the conv matmuls arrive (depends only on wd_sb) ----
            warm = pre_psum.tile([128, 128], dt.float32, tag="pp")
            for i in range(32):
                nc.tensor.matmul(
                    warm[:],
                    lhsT=wd_sb[:, 0, :, :],
                    rhs=wd_sb[:, 1, :, :],
                    start=True,
                    stop=True,
                    skip_group_check=True,
                )

            # ---- prologue: s = softplus(ltnt @ Wd + bd) + 1, per (half, c) ----
            s_pre = pre_psum.tile([128, 1], dt.float32, tag="pp")
            for a in range(2):
                nc.tensor.matmul(
                    s_pre[:],
                    lhsT=wd_sb[:, a, :, :],
                    rhs=lt_sb[:, a : a + 1],
                    start=(a == 0),
                    stop=(a == 1),
                )
            # softplus(p) = ln(1 + exp(p)); only exp/ln/copy exist in one ACT set
            e_sb = pre.tile([128, 1], dt.float32, tag="e")
            nc.scalar.activation(
                e_sb[:], s_pre[:], mybir.ActivationFunctionType.Exp, bias=bd_sb[:]
            )
            nc.vector.tensor_scalar_add(e_sb[:], e_sb[:], 1.0)
            s_sb = pre.tile([128, 1], dt.float32, tag="s")
            nc.scalar.activation(s_sb[:], e_sb[:], mybir.ActivationFunctionType.Ln)
            nc.vector.tensor_scalar_add(s_sb[:], s_sb[:], 1.0)
            s2_sb = pre.tile([128, 1], dt.float32, tag="s2")
            nc.vector.tensor_mul(s2_sb[:], s_sb[:], s_sb[:])

            # ---- conv weights: w'[(h,c),(k,f)] = K[k,c,f] * s[c], bf16.
            # (the demodulation d[f] is applied per-partition at PSUM drain,
            # keeping it OFF the critical path to the first conv matmul) ----
            wfin = pre.tile([128, KW * F], dt.bfloat16, tag="wfin")
            nc.vector.tensor_scalar_mul(wfin[:], ker_flat[:], s_sb[:])

            # ---- d = 1/sqrt(sum_kc K^2 s^2 + eps) as [128=(h,f), 1] ----
            k2_sb = pre.tile([128, KW * F], dt.float32, tag="k2")
            nc.vector.tensor_mul(k2_sb[:], ker_flat[:], ker_flat[:])
            dpre = pre_psum.tile([128, 1], dt.float32, tag="pp")
            for h in range(2):
                lo, hi = h * 64, (h + 1) * 64
                for k in range(KW):
                    nc.tensor.matmul(
                        dpre[lo:hi, :],
                        lhsT=k2_sb[lo:hi, k * F : (k + 1) * F],
                        rhs=s2_sb[lo:hi, :],
                        start=(k == 0),
                        stop=(k == KW - 1),
                        skip_group_check=True,
                    )
            # rsqrt(v) = exp(-0.5 * ln(v)) — keeps ACT funcs within one LUT set
            eps_sb = pre.tile([128, 1], dt.float32, tag="eps")
            nc.vector.memset(eps_sb[:], EPS)
            lnv = pre.tile([128, 1], dt.float32, tag="lnv")
            nc.scalar.activation(
                lnv[:], dpre[:], mybir.ActivationFunctionType.Ln, bias=eps_sb[:]
            )
            dvec = pre.tile([128, 1], dt.float32, tag="dvec")
            nc.scalar.activation(
                dvec[:], lnv[:], mybir.ActivationFunctionType.Exp, scale=-0.5
            )

            # ---- main conv loop ----
            yc = [
                yout_pool.tile(
                    [128, CHW], dt.bfloat16, name=f"yout{c}", tag=f"yout{c}"
                )
                for c in range(NCHUNK)
            ]
            for g in range(NGRP):
                c, j = divmod(g, NGRP // NCHUNK)
                base = j * 512
                ps = conv_psum.tile([128, 512], dt.float32, tag="convps")
                for k in range(KW):
                    st, sp = (k == 0), (k == KW - 1)
                    for h in range(2):
                        nc.tensor.matmul(
                            ps[h * 64 : (h + 1) * 64, :],
                            lhsT=wfin[h * 64 : (h + 1) * 64, k * F : (k + 1) * F],
                            rhs=xc[c][h * 64 : (h + 1) * 64, base + k : base + k + 512],
                            start=st,
                            stop=sp,
                            skip_group_check=True,
                        )
                dst = yc[c][:, base : base + 512]
                if g % 2 == 0:
                    nc.vector.tensor_scalar_mul(dst, ps[:], dvec[:])
                else:
                    nc.scalar.activation(
                        dst, ps[:], mybir.ActivationFunctionType.Copy, scale=dvec[:]
                    )
                if j == NGRP // NCHUNK - 1:
                    nc.scalar.dma_start(
                        out=yout[:, c * CHW : (c + 1) * CHW], in_=yc[c][:]
                    )

    nc.compile()
    return nc


def _get_nc():
    if "nc" not in _cached:
        _cached["nc"] = _build()
    return _cached["nc"]


def kernel(data, ltnt, kernel, Wd, bd):
    from concourse import bass_utils

    nc = _get_nc()

    data = np.asarray(data, dtype=np.float32)
    ltnt = np.asarray(ltnt, dtype=np.float32)
    kf = np.ascontiguousarray(np.asarray(kernel, dtype=np.float32))
    wdf = np.ascontiguousarray(np.asarray(Wd, dtype=np.float32))
    bdf = np.ascontiguousarray(np.asarray(bd, dtype=np.float32))

    in_maps = []
    for b in range(B):
        xt = data[b].reshape(2, H, C).transpose(0, 2, 1)  # [2, C, H]
        xin = np.zeros((128, H + 2), dtype=BF16)
        xin[:, 1 : H + 1] = xt.reshape(128, H).astype(BF16)
        xin[64:128, 0] = xt[0, :, -1].astype(BF16)  # x[H-1] left halo of half 1
        xin[0:64, H + 1] = xt[1, :, 0].astype(BF16)  # x[H] right halo of half 0
        in_maps.append(
            {
                "xin": xin,
                "lt": np.ascontiguousarray(ltnt[b]),
                "ker": kf,
                "wd": wdf,
                "bd": bdf,
            }
        )

    res = bass_utils.run_bass_kernel_spmd(nc, in_maps, core_ids=list(range(B)))

    out = np.empty((B, L, C), dtype=np.float32)
    for b in range(B):
        yo = np.asarray(res.results[b]["yout"])  # [128, H] bf16
        out[b] = (
            yo.reshape(2, F, H).transpose(0, 2, 1).reshape(L, F).astype(np.float32)
        )
    return out


# revision 11
# speedup vs baseline: 1.0924x; 1.0722x over previous
"""AdaModConv1D on 8 TRN2 NeuronCores — pure data parallel (1 sample/core).

Math: s = softplus(ltnt @ Wd + bd) + 1          [B, C]
      d = rsqrt(einsum('kcf,bc->bf', K^2, s^2) + eps)
      y = conv1d(x * s, K, SAME) * d

Per-core trick: each core owns ONE sample, so the input modulation (x*s) and
output demodulation (y*d) fold into the conv weights:
      w''[k,c,f] = K[k,c,f] * s[c] * d[f];   y = conv1d(x, w'')
making the device inner loop a pure 3-tap conv1d = 3 accumulating matmuls
per output tile on the PE, with a tiny on-device prologue computing w''.

Layout: host pre-transposes each sample to channels-first bf16
[128 = (half, c), 32770] with one halo column on each side (half = L split in
two so both partition halves are used; the two 64x64 conv quadrants run at
PE tile_position (0,0) and (64,64) concurrently). Output comes back as
y^T [128 = (half, f), 32768] bf16 and is re-transposed on host.
"""

import os
import sys

sys.path.insert(0, "/opt/trn_rl_repo")

import numpy as np
import ml_dtypes

BF16 = ml_dtypes.bfloat16

B, L, C = 8, 65536, 64
F, KW, DL = 64, 3, 256
EPS = 1e-8
H = L // 2          # 32768, half length per partition group
NCHUNK = 8          # DMA chunks per direction
CHW = H // NCHUNK   # 4096 columns per chunk
NGRP = H // 512     # 64 matmul groups of 512 outputs x 2 halves
PARW = 256 + 2 + 1 + 192  # packed params width

_cached = {}


def _build():
    import concourse.bass as bass
    import concourse.bacc as bacc
    import concourse.mybir as mybir
    import concourse.tile as tile

    dt = mybir.dt
    nc = bacc.Bacc("TRN2", target_bir_lowering=False, debug=False, num_devices=8)

    xin = nc.declare_dram_parameter("xin", [128, H + 2], dt.bfloat16, isOutput=False)
    # host-packed params, one DMA: [wd_dup 256 | lt 2 | bd_dup 1 | ker 192]
    par = nc.declare_dram_parameter("par", [128, PARW], dt.float32, isOutput=False)
    yout = nc.declare_dram_parameter("yout", [128, H], dt.bfloat16, isOutput=True)

    with tile.TileContext(nc) as tc:
        with (
            tc.tile_pool(name="xin", bufs=1) as xin_pool,
            tc.tile_pool(name="yout", bufs=1) as yout_pool,
            tc.tile_pool(name="pre", bufs=1) as pre,
            tc.tile_pool(name="pp", bufs=2, space="PSUM") as pre_psum,
            tc.tile_pool(name="cp", bufs=6, space="PSUM") as conv_psum,
        ):
            # ---- input chunk DMAs (issued first so they stream early) ----
            xc = []
            for c in range(NCHUNK):
                t = xin_pool.tile([128, CHW + 2], dt.bfloat16, tag=f"xin{c}")
                nc.sync.dma_start(out=t[:], in_=xin[:, c * CHW : c * CHW + CHW + 2])
                xc.append(t)

            # ---- prologue: one packed param DMA (before the chunk DMAs
            # in the same sync-ring FIFO would be wrong; this is issued
            # first so it lands first) ----
            par_sb = pre.tile([128, PARW], dt.float32, tag="par")
            nc.sync.dma_start(out=par_sb[:], in_=par[:])
            lt_sb = par_sb[:, 256:258]
            bd_sb = par_sb[:, 258:259]
            ker_flat = par_sb[:, 259 : 259 + KW * F]

            # ---- prologue: s = softplus(ltnt @ Wd + bd) + 1, per (half, c) ----
            s_pre = pre_psum.tile([128, 1], dt.float32, tag="pp")
            for a in range(2):
                nc.tensor.matmul(
                    s_pre[:],
                    lhsT=par_sb[:, a * 128 : (a + 1) * 128],
                    rhs=lt_sb[:, a : a + 1],
                    start=(a == 0),
                    stop=(a == 1),
                )
            # softplus(p) = ln(1 + exp(p)); only exp/ln/copy exist in one ACT set
            e_sb = pre.tile([128, 1], dt.float32, tag="e")
            nc.scalar.activation(
                e_sb[:], s_pre[:], mybir.ActivationFunctionType.Exp, bias=bd_sb[:]
            )
            nc.vector.tensor_scalar_add(e_sb[:], e_sb[:], 1.0)
            s_sb = pre.tile([128, 1], dt.float32, tag="s")
            nc.scalar.activation(s_sb[:], e_sb[:], mybir.ActivationFunctionType.Ln)
            nc.vector.tensor_scalar_add(s_sb[:], s_sb[:], 1.0)
            s2_sb = pre.tile([128, 1], dt.float32, tag="s2")
            nc.vector.tensor_mul(s2_sb[:], s_sb[:], s_sb[:])

            # ---- conv weights: w'[(h,c),(k,f)] = K[k,c,f] * s[c], bf16.
            # (the demodulation d[f] is applied per-partition at PSUM drain,
            # keeping it OFF the critical path to the first conv matmul) ----
            wfin = pre.tile([128, KW * F], dt.bfloat16, tag="wfin")
            nc.vector.tensor_scalar_mul(wfin[:], ker_flat[:], s_sb[:])

            # ---- d = 1/sqrt(sum_kc K^2 s^2 + eps) as [128=(h,f), 1] ----
            k2_sb = pre.tile([128, KW * F], dt.float32, tag="k2")
            nc.vector.tensor_mul(k2_sb[:], ker_flat[:], ker_flat[:])
            dpre = pre_psum.tile([128, 1], dt.float32, tag="pp")
            for h in range(2):
                lo, hi = h * 64, (h + 1) * 64
                for k in range(KW):
                    nc.tensor.matmul(
                        dpre[lo:hi, :],
                        lhsT=k2_sb[lo:hi, k * F : (k + 1) * F],
                        rhs=s2_sb[lo:hi, :],
                        start=(k == 0),
                        stop=(k == KW - 1),
                        skip_group_check=True,
                    )
            # rsqrt(v) = exp(-0.5 * ln(v)) — keeps ACT funcs within one LUT set
            eps_sb = pre.tile([128, 1], dt.float32, tag="eps")
            nc.vector.memset(eps_sb[:], EPS)
            lnv = pre.tile([128, 1], dt.float32, tag="lnv")
            nc.scalar.activation(
                lnv[:], dpre[:], mybir.ActivationFunctionType.Ln, bias=eps_sb[:]
            )
            dvec = pre.tile([128, 1], dt.float32, tag="dvec")
            nc.scalar.activation(
                dvec[:], lnv[:], mybir.ActivationFunctionType.Exp, scale=-0.5
            )

            # ---- main conv loop ----
            yc = [
                yout_pool.tile(
                    [128, CHW], dt.bfloat16, name=f"yout{c}", tag=f"yout{c}"
                )
                for c in range(NCHUNK)
            ]
            for g in range(NGRP):
                c, j = divmod(g, NGRP // NCHUNK)
                base = j * 512
                ps = conv_psum.tile([128, 512], dt.float32, tag="convps")
                for k in range(KW):
                    st, sp = (k == 0), (k == KW - 1)
                    for h in range(2):
                        nc.tensor.matmul(
                            ps[h * 64 : (h + 1) * 64, :],
                            lhsT=wfin[h * 64 : (h + 1) * 64, k * F : (k + 1) * F],
                            rhs=xc[c][h * 64 : (h + 1) * 64, base + k : base + k + 512],
                            start=st,
                            stop=sp,
                            skip_group_check=True,
                        )
                dst = yc[c][:, base : base + 512]
                if g % 2 == 0:
                    nc.vector.tensor_scalar_mul(dst, ps[:], dvec[:])
                else:
                    nc.scalar.activation(
                        dst, ps[:], mybir.ActivationFunctionType.Copy, scale=dvec[:]
                    )
                if j == NGRP // NCHUNK - 1:
                    nc.scalar.dma_start(
                        out=yout[:, c * CHW : (c + 1) * CHW], in_=yc[c][:]
                    )

    nc.compile()
    return nc


def _get_nc():
    if "nc" not in _cached:
        _cached["nc"] = _build()
    return _cached["nc"]


def pack_params(ltnt_b, kernel, Wd, bd):
    """[128, PARW] f32: wd dup'd per half (2 x [128,128]), lt, bd dup, ker."""
    par = np.empty((128, PARW), dtype=np.float32)
    for a in range(2):
        par[:, a * 128 : a * 128 + 64] = Wd[a * 128 : (a + 1) * 128, :]
        par[:, a * 128 + 64 : (a + 1) * 128] = Wd[a * 128 : (a + 1) * 128, :]
    par[:, 256] = ltnt_b[0:128]
    par[:, 257] = ltnt_b[128:256]
    par[:, 258] = np.tile(bd, 2)
    # ker block: par[p, 259 + k*64 + f] = kernel[k, p % 64, f]
    kblk = kernel.transpose(1, 0, 2).reshape(64, KW * F)  # [c, (k,f)]
    par[:, 259:] = np.tile(kblk, (2, 1))
    return par


def make_xin(data_b):
    xt = data_b.reshape(2, H, C).transpose(0, 2, 1)  # [2, C, H]
    xin = np.zeros((128, H + 2), dtype=BF16)
    xin[:, 1 : H + 1] = xt.reshape(128, H).astype(BF16)
    xin[64:128, 0] = xt[0, :, -1].astype(BF16)  # x[H-1] left halo of half 1
    xin[0:64, H + 1] = xt[1, :, 0].astype(BF16)  # x[H] right halo of half 0
    return xin


def kernel(data, ltnt, kernel, Wd, bd):
    from concourse import bass_utils

    nc = _get_nc()

    data = np.asarray(data, dtype=np.float32)
    ltnt = np.asarray(ltnt, dtype=np.float32)
    kf = np.asarray(kernel, dtype=np.float32)
    wdf = np.asarray(Wd, dtype=np.float32)
    bdf = np.asarray(bd, dtype=np.float32)

    in_maps = [
        {"xin": make_xin(data[b]), "par": pack_params(ltnt[b], kf, wdf, bdf)}
        for b in range(B)
    ]

    res = bass_utils.run_bass_kernel_spmd(nc, in_maps, core_ids=list(range(B)))

    out = np.empty((B, L, C), dtype=np.float32)
    for b in range(B):
        yo = np.asarray(res.results[b]["yout"])  # [128, H] bf16
        out[b] = (
            yo.reshape(2, F, H).transpose(0, 2, 1).reshape(L, F).astype(np.float32)
        )
    return out


# revision 12
# speedup vs baseline: 1.4658x; 1.3418x over previous
"""AdaModConv1D on 8 TRN2 NeuronCores — pure data parallel (1 sample/core).

Math: s = softplus(ltnt @ Wd + bd) + 1          [B, C]
      d = rsqrt(einsum('kcf,bc->bf', K^2, s^2) + eps)
      y = conv1d(x * s, K, SAME) * d

Per-core trick: each core owns ONE sample, so the input modulation (x*s) and
output demodulation (y*d) fold into the conv weights:
      w''[k,c,f] = K[k,c,f] * s[c] * d[f];   y = conv1d(x, w'')
making the device inner loop a pure 3-tap conv1d = 3 accumulating matmuls
per output tile on the PE, with a tiny on-device prologue computing w''.

Layout: host pre-transposes each sample to channels-first bf16
[128 = (half, c), 32770] with one halo column on each side (half = L split in
two so both partition halves are used; the two 64x64 conv quadrants run at
PE tile_position (0,0) and (64,64) concurrently). Output comes back as
y^T [128 = (half, f), 32768] bf16 and is re-transposed on host.
"""

import os
import sys

sys.path.insert(0, "/opt/trn_rl_repo")

import numpy as np
import ml_dtypes

BF16 = ml_dtypes.bfloat16

B, L, C = 8, 65536, 64
F, KW, DL = 64, 3, 256
EPS = 1e-8
H = L // 2          # 32768, half length per partition group
NCHUNK = 8          # DMA chunks per direction
CHW = H // NCHUNK   # 4096 columns per chunk
NGRP = H // 512     # 64 matmul groups of 512 outputs x 2 halves
PARW = 256 + 2 + 1 + 192  # packed params width

_cached = {}


def _build():
    import concourse.bass as bass
    import concourse.bacc as bacc
    import concourse.mybir as mybir
    import concourse.tile as tile

    dt = mybir.dt
    nc = bacc.Bacc("TRN2", target_bir_lowering=False, debug=False, num_devices=8)

    xin = nc.declare_dram_parameter("xin", [128, H + 2], dt.bfloat16, isOutput=False)
    # host-packed params, one DMA: [wd_dup 256 | lt 2 | bd_dup 1 | ker 192]
    par = nc.declare_dram_parameter("par", [128, PARW], dt.float32, isOutput=False)
    yout = nc.declare_dram_parameter("yout", [128, H], dt.bfloat16, isOutput=True)

    with tile.TileContext(nc) as tc:
        with (
            tc.tile_pool(name="xin", bufs=1) as xin_pool,
            tc.tile_pool(name="yout", bufs=1) as yout_pool,
            tc.tile_pool(name="pre", bufs=1) as pre,
            tc.tile_pool(name="pp", bufs=2, space="PSUM") as pre_psum,
            tc.tile_pool(name="cp", bufs=6, space="PSUM") as conv_psum,
        ):
            # ---- ACT table prewarm: dummy exp/ln/copy with no DMA deps so
            # all three LUT segments load during the DMA window, not inside
            # the s-chain critical path ----
            eps_sb = pre.tile([128, 1], dt.float32, tag="eps")
            nc.vector.memset(eps_sb[:], EPS)
            scr = pre.tile([128, 1], dt.float32, tag="scr")
            nc.scalar.activation(scr[:], eps_sb[:], mybir.ActivationFunctionType.Exp)
            nc.scalar.activation(scr[:], eps_sb[:], mybir.ActivationFunctionType.Ln)
            nc.scalar.activation(scr[:], eps_sb[:], mybir.ActivationFunctionType.Copy)

            # ---- packed param DMA: MUST be issued before the chunk DMAs —
            # same HWDGE FIFO lane, completion milestones are cumulative ----
            par_sb = pre.tile([128, PARW], dt.float32, tag="par")
            nc.sync.dma_start(out=par_sb[:], in_=par[:])
            lt_sb = par_sb[:, 256:258]
            bd_sb = par_sb[:, 258:259]
            ker_flat = par_sb[:, 259 : 259 + KW * F]

            # ---- input chunk DMAs ----
            xc = []
            for c in range(NCHUNK):
                t = xin_pool.tile([128, CHW + 2], dt.bfloat16, tag=f"xin{c}")
                nc.sync.dma_start(out=t[:], in_=xin[:, c * CHW : c * CHW + CHW + 2])
                xc.append(t)

            # ---- prologue: s = softplus(ltnt @ Wd + bd) + 1, per (half, c) ----
            s_pre = pre_psum.tile([128, 1], dt.float32, tag="pp")
            for a in range(2):
                nc.tensor.matmul(
                    s_pre[:],
                    lhsT=par_sb[:, a * 128 : (a + 1) * 128],
                    rhs=lt_sb[:, a : a + 1],
                    start=(a == 0),
                    stop=(a == 1),
                )
            # softplus(p) = ln(1 + exp(p)); only exp/ln/copy exist in one ACT set
            e_sb = pre.tile([128, 1], dt.float32, tag="e")
            nc.scalar.activation(
                e_sb[:], s_pre[:], mybir.ActivationFunctionType.Exp, bias=bd_sb[:]
            )
            nc.vector.tensor_scalar_add(e_sb[:], e_sb[:], 1.0)
            s_sb = pre.tile([128, 1], dt.float32, tag="s")
            nc.scalar.activation(s_sb[:], e_sb[:], mybir.ActivationFunctionType.Ln)
            nc.vector.tensor_scalar_add(s_sb[:], s_sb[:], 1.0)
            s2_sb = pre.tile([128, 1], dt.float32, tag="s2")
            nc.vector.tensor_mul(s2_sb[:], s_sb[:], s_sb[:])

            # ---- conv weights: w'[(h,c),(k,f)] = K[k,c,f] * s[c], bf16.
            # (the demodulation d[f] is applied per-partition at PSUM drain,
            # keeping it OFF the critical path to the first conv matmul) ----
            wfin = pre.tile([128, KW * F], dt.bfloat16, tag="wfin")
            nc.vector.tensor_scalar_mul(wfin[:], ker_flat[:], s_sb[:])

            # ---- d = 1/sqrt(sum_kc K^2 s^2 + eps) as [128=(h,f), 1] ----
            k2_sb = pre.tile([128, KW * F], dt.float32, tag="k2")
            nc.vector.tensor_mul(k2_sb[:], ker_flat[:], ker_flat[:])
            dpre = pre_psum.tile([128, 1], dt.float32, tag="pp")
            for h in range(2):
                lo, hi = h * 64, (h + 1) * 64
                for k in range(KW):
                    nc.tensor.matmul(
                        dpre[lo:hi, :],
                        lhsT=k2_sb[lo:hi, k * F : (k + 1) * F],
                        rhs=s2_sb[lo:hi, :],
                        start=(k == 0),
                        stop=(k == KW - 1),
                        skip_group_check=True,
                    )
            # rsqrt(v) = exp(-0.5 * ln(v)) — keeps ACT funcs within one LUT set
            lnv = pre.tile([128, 1], dt.float32, tag="lnv")
            nc.scalar.activation(
                lnv[:], dpre[:], mybir.ActivationFunctionType.Ln, bias=eps_sb[:]
            )
            dvec = pre.tile([128, 1], dt.float32, tag="dvec")
            nc.scalar.activation(
                dvec[:], lnv[:], mybir.ActivationFunctionType.Exp, scale=-0.5
            )

            # ---- main conv loop ----
            yc = [
                yout_pool.tile(
                    [128, CHW], dt.bfloat16, name=f"yout{c}", tag=f"yout{c}"
                )
                for c in range(NCHUNK)
            ]
            for g in range(NGRP):
                c, j = divmod(g, NGRP // NCHUNK)
                base = j * 512
                ps = conv_psum.tile([128, 512], dt.float32, tag="convps")
                for k in range(KW):
                    st, sp = (k == 0), (k == KW - 1)
                    for h in range(2):
                        nc.tensor.matmul(
                            ps[h * 64 : (h + 1) * 64, :],
                            lhsT=wfin[h * 64 : (h + 1) * 64, k * F : (k + 1) * F],
                            rhs=xc[c][h * 64 : (h + 1) * 64, base + k : base + k + 512],
                            start=st,
                            stop=sp,
                            skip_group_check=True,
                        )
                dst = yc[c][:, base : base + 512]
                if g % 2 == 0:
                    nc.vector.tensor_scalar_mul(dst, ps[:], dvec[:])
                else:
                    nc.scalar.activation(
                        dst, ps[:], mybir.ActivationFunctionType.Copy, scale=dvec[:]
                    )
                if j % 4 == 3:
                    p0 = (j - 3) * 512
                    nc.sync.dma_start(
                        out=yout[:, c * CHW + p0 : c * CHW + p0 + 2048],
                        in_=yc[c][:, p0 : p0 + 2048],
                    )

    nc.compile()
    return nc


def _get_nc():
    if "nc" not in _cached:
        _cached["nc"] = _build()
    return _cached["nc"]


def pack_params(ltnt_b, kernel, Wd, bd):
    """[128, PARW] f32: wd dup'd per half (2 x [128,128]), lt, bd dup, ker."""
    par = np.empty((128, PARW), dtype=np.float32)
    for a in range(2):
        par[:, a * 128 : a * 128 + 64] = Wd[a * 128 : (a + 1) * 128, :]
        par[:, a * 128 + 64 : (a + 1) * 128] = Wd[a * 128 : (a + 1) * 128, :]
    par[:, 256] = ltnt_b[0:128]
    par[:, 257] = ltnt_b[128:256]
    par[:, 258] = np.tile(bd, 2)
    # ker block: par[p, 259 + k*64 + f] = kernel[k, p % 64, f]
    kblk = kernel.transpose(1, 0, 2).reshape(64, KW * F)  # [c, (k,f)]
    par[:, 259:] = np.tile(kblk, (2, 1))
    return par


def make_xin(data_b):
    xt = data_b.reshape(2, H, C).transpose(0, 2, 1)  # [2, C, H]
    xin = np.zeros((128, H + 2), dtype=BF16)
    xin[:, 1 : H + 1] = xt.reshape(128, H).astype(BF16)
    xin[64:128, 0] = xt[0, :, -1].astype(BF16)  # x[H-1] left halo of half 1
    xin[0:64, H + 1] = xt[1, :, 0].astype(BF16)  # x[H] right halo of half 0
    return xin


def kernel(data, ltnt, kernel, Wd, bd):
    from concourse import bass_utils

    nc = _get_nc()

    data = np.asarray(data, dtype=np.float32)
    ltnt = np.asarray(ltnt, dtype=np.float32)
    kf = np.asarray(kernel, dtype=np.float32)
    wdf = np.asarray(Wd, dtype=np.float32)
    bdf = np.asarray(bd, dtype=np.float32)

    in_maps = [
        {"xin": make_xin(data[b]), "par": pack_params(ltnt[b], kf, wdf, bdf)}
        for b in range(B)
    ]

    res = bass_utils.run_bass_kernel_spmd(nc, in_maps, core_ids=list(range(B)))

    out = np.empty((B, L, C), dtype=np.float32)
    for b in range(B):
        yo = np.asarray(res.results[b]["yout"])  # [128, H] bf16
        out[b] = (
            yo.reshape(2, F, H).transpose(0, 2, 1).reshape(L, F).astype(np.float32)
        )
    return out


# revision 14
# speedup vs baseline: 1.6543x; 1.1286x over previous
"""AdaModConv1D on 8 TRN2 NeuronCores — pure data parallel (1 sample/core).

Math: s = softplus(ltnt @ Wd + bd) + 1          [B, C]
      d = rsqrt(einsum('kcf,bc->bf', K^2, s^2) + eps)
      y = conv1d(x * s, K, SAME) * d

Per-core trick: each core owns ONE sample, so the input modulation (x*s) and
output demodulation (y*d) fold into the conv weights:
      w''[k,c,f] = K[k,c,f] * s[c] * d[f];   y = conv1d(x, w'')
making the device inner loop a pure 3-tap conv1d = 3 accumulating matmuls
per output tile on the PE, with a tiny on-device prologue computing w''.

Layout: host pre-transposes each sample to channels-first bf16
[128 = (half, c), 32770] with one halo column on each side (half = L split in
two so both partition halves are used; the two 64x64 conv quadrants run at
PE tile_position (0,0) and (64,64) concurrently). Output comes back as
y^T [128 = (half, f), 32768] bf16 and is re-transposed on host.
"""

import os
import sys

sys.path.insert(0, "/opt/trn_rl_repo")

import numpy as np
import ml_dtypes

BF16 = ml_dtypes.bfloat16

B, L, C = 8, 65536, 64
F, KW, DL = 64, 3, 256
EPS = 1e-8
H = L // 2          # 32768, half length per partition group
NCHUNK = 8          # DMA chunks per direction
CHW = H // NCHUNK   # 4096 columns per chunk
NGRP = H // 512     # 64 matmul groups of 512 outputs x 2 halves
PARW = 256 + 2 + 1 + 192  # packed params width

_cached = {}


def _build():
    import concourse.bass as bass
    import concourse.bacc as bacc
    import concourse.mybir as mybir
    import concourse.tile as tile

    dt = mybir.dt
    nc = bacc.Bacc("TRN2", target_bir_lowering=False, debug=False, num_devices=8)

    xin = nc.declare_dram_parameter("xin", [128, H + 2], dt.bfloat16, isOutput=False)
    # host-packed params, one DMA: [wd_dup 256 | lt 2 | bd_dup 1 | ker 192]
    par = nc.declare_dram_parameter("par", [128, PARW], dt.float32, isOutput=False)
    yout = nc.declare_dram_parameter("yout", [128, H], dt.bfloat16, isOutput=True)

    with tile.TileContext(nc) as tc:
        with (
            tc.tile_pool(name="xin", bufs=1) as xin_pool,
            tc.tile_pool(name="yout", bufs=1) as yout_pool,
            tc.tile_pool(name="pre", bufs=1) as pre,
            tc.tile_pool(name="pp", bufs=2, space="PSUM") as pre_psum,
            tc.tile_pool(name="cp", bufs=6, space="PSUM") as conv_psum,
        ):
            # ---- ACT table prewarm: dummy exp/ln/copy with no DMA deps so
            # all three LUT segments load during the DMA window, not inside
            # the s-chain critical path ----
            eps_sb = pre.tile([128, 1], dt.float32, tag="eps")
            nc.vector.memset(eps_sb[:], EPS)
            scr = pre.tile([128, 1], dt.float32, tag="scr")
            nc.scalar.activation(scr[:], eps_sb[:], mybir.ActivationFunctionType.Exp)
            nc.scalar.activation(scr[:], eps_sb[:], mybir.ActivationFunctionType.Ln)
            nc.scalar.activation(scr[:], eps_sb[:], mybir.ActivationFunctionType.Copy)

            # ---- packed param DMA: MUST be issued before the chunk DMAs —
            # same HWDGE FIFO lane, completion milestones are cumulative ----
            par_sb = pre.tile([128, PARW], dt.float32, tag="par")
            nc.sync.dma_start(out=par_sb[:], in_=par[:])
            lt_sb = par_sb[:, 256:258]
            bd_sb = par_sb[:, 258:259]
            ker_flat = par_sb[:, 259 : 259 + KW * F]

            # ---- input chunk DMAs ----
            xc = []
            for c in range(NCHUNK):
                t = xin_pool.tile([128, CHW + 2], dt.bfloat16, tag=f"xin{c}")
                nc.sync.dma_start(out=t[:], in_=xin[:, c * CHW : c * CHW + CHW + 2])
                xc.append(t)

            # ---- prologue: s = softplus(ltnt @ Wd + bd) + 1, per (half, c) ----
            s_pre = pre_psum.tile([128, 1], dt.float32, tag="pp")
            for a in range(2):
                nc.tensor.matmul(
                    s_pre[:],
                    lhsT=par_sb[:, a * 128 : (a + 1) * 128],
                    rhs=lt_sb[:, a : a + 1],
                    start=(a == 0),
                    stop=(a == 1),
                )
            # softplus(p) = ln(1 + exp(p)); only exp/ln/copy exist in one ACT set
            e_sb = pre.tile([128, 1], dt.float32, tag="e")
            nc.scalar.activation(
                e_sb[:], s_pre[:], mybir.ActivationFunctionType.Exp, bias=bd_sb[:]
            )
            nc.vector.tensor_scalar_add(e_sb[:], e_sb[:], 1.0)
            s_sb = pre.tile([128, 1], dt.float32, tag="s")
            nc.scalar.activation(s_sb[:], e_sb[:], mybir.ActivationFunctionType.Ln)
            nc.vector.tensor_scalar_add(s_sb[:], s_sb[:], 1.0)
            s2_sb = pre.tile([128, 1], dt.float32, tag="s2")
            nc.vector.tensor_mul(s2_sb[:], s_sb[:], s_sb[:])

            # ---- d = 1/sqrt(sum_kc K^2 s^2 + eps) as [1, F] ----
            k2_sb = pre.tile([128, KW * F], dt.float32, tag="k2")
            nc.vector.tensor_mul(k2_sb[:], ker_flat[:], ker_flat[:])
            dpre = pre_psum.tile([1, F], dt.float32, tag="pp")
            for k in range(KW):
                nc.tensor.matmul(
                    dpre[:],
                    lhsT=s2_sb[0:64, :],
                    rhs=k2_sb[0:64, k * F : (k + 1) * F],
                    start=(k == 0),
                    stop=(k == KW - 1),
                )
            # rsqrt(v) = exp(-0.5 * ln(v)) — keeps ACT funcs within one LUT set
            lnv = pre.tile([1, F], dt.float32, tag="lnv")
            nc.scalar.activation(
                lnv[:], dpre[:], mybir.ActivationFunctionType.Ln, bias=eps_sb[0:1, :]
            )
            d_sb = pre.tile([1, F], dt.float32, tag="d")
            nc.scalar.activation(
                d_sb[:], lnv[:], mybir.ActivationFunctionType.Exp, scale=-0.5
            )
            d3_sb = pre.tile([1, KW * F], dt.float32, tag="d3")
            for k in range(KW):
                nc.vector.tensor_copy(d3_sb[:, k * F : (k + 1) * F], d_sb[:])

            # ---- w''[(h,c),(k,f)] = K[k,c,f] * d[f] * s[c], bf16 ----
            ones = pre.tile([1, 64], dt.float32, tag="ones")
            nc.vector.memset(ones[:], 1.0)
            dmat = pre_psum.tile([128, KW * F], dt.float32, tag="pp")
            for h in range(2):
                nc.tensor.matmul(
                    dmat[h * 64 : (h + 1) * 64, :],
                    lhsT=ones[:],
                    rhs=d3_sb[:],
                    start=True,
                    stop=True,
                )
            wtmp = pre.tile([128, KW * F], dt.float32, tag="wtmp")
            nc.vector.tensor_mul(wtmp[:], ker_flat[:], dmat[:])
            wfin = pre.tile([128, KW * F], dt.bfloat16, tag="wfin")
            nc.vector.tensor_scalar_mul(wfin[:], wtmp[:], s_sb[:])

            # ---- main conv loop: 4 concurrent 64x64 PE quadrants per tap,
            # two 512-col windows (W0->bank X normal layout, W1->bank Y with
            # partition halves swapped; the host unswizzles odd windows) ----
            yc = [
                yout_pool.tile(
                    [128, CHW], dt.bfloat16, name=f"yout{c}", tag=f"yout{c}"
                )
                for c in range(NCHUNK)
            ]
            for gp in range(NGRP // 2):
                c = gp // 4
                j0 = (gp % 4) * 2
                b0, b1 = j0 * 512, (j0 + 1) * 512
                psX = conv_psum.tile([128, 512], dt.float32, name="psX", tag="convps")
                psY = conv_psum.tile([128, 512], dt.float32, name="psY", tag="convps")
                x = xc[c]
                for k in range(KW):
                    st, sp = (k == 0), (k == KW - 1)
                    wA = wfin[0:64, k * F : (k + 1) * F]
                    wB = wfin[64:128, k * F : (k + 1) * F]
                    nc.tensor.matmul(
                        psX[0:64, :], lhsT=wA, rhs=x[0:64, b0 + k : b0 + k + 512],
                        start=st, stop=sp, skip_group_check=True,
                    )
                    nc.tensor.matmul(
                        psX[64:128, :], lhsT=wB, rhs=x[64:128, b0 + k : b0 + k + 512],
                        start=st, stop=sp, skip_group_check=True,
                    )
                    nc.tensor.matmul(
                        psY[64:128, :], lhsT=wA, rhs=x[0:64, b1 + k : b1 + k + 512],
                        start=st, stop=sp, skip_group_check=True,
                    )
                    nc.tensor.matmul(
                        psY[0:64, :], lhsT=wB, rhs=x[64:128, b1 + k : b1 + k + 512],
                        start=st, stop=sp, skip_group_check=True,
                    )
                if gp % 2 == 0:
                    nc.vector.tensor_copy(yc[c][:, b0 : b0 + 512], psX[:])
                    nc.scalar.copy(yc[c][:, b1 : b1 + 512], psY[:])
                else:
                    nc.scalar.copy(yc[c][:, b0 : b0 + 512], psX[:])
                    nc.vector.tensor_copy(yc[c][:, b1 : b1 + 512], psY[:])
                if gp % 2 == 1:
                    p0 = (j0 - 2) * 512
                    nc.sync.dma_start(
                        out=yout[:, c * CHW + p0 : c * CHW + p0 + 2048],
                        in_=yc[c][:, p0 : p0 + 2048],
                    )

    nc.compile()
    return nc


def _get_nc():
    if "nc" not in _cached:
        _cached["nc"] = _build()
    return _cached["nc"]


def pack_params(ltnt_b, kernel, Wd, bd):
    """[128, PARW] f32: wd dup'd per half (2 x [128,128]), lt, bd dup, ker."""
    par = np.empty((128, PARW), dtype=np.float32)
    for a in range(2):
        par[:, a * 128 : a * 128 + 64] = Wd[a * 128 : (a + 1) * 128, :]
        par[:, a * 128 + 64 : (a + 1) * 128] = Wd[a * 128 : (a + 1) * 128, :]
    par[:, 256] = ltnt_b[0:128]
    par[:, 257] = ltnt_b[128:256]
    par[:, 258] = np.tile(bd, 2)
    # ker block: par[p, 259 + k*64 + f] = kernel[k, p % 64, f]
    kblk = kernel.transpose(1, 0, 2).reshape(64, KW * F)  # [c, (k,f)]
    par[:, 259:] = np.tile(kblk, (2, 1))
    return par


def make_xin(data_b):
    xt = data_b.reshape(2, H, C).transpose(0, 2, 1)  # [2, C, H]
    xin = np.zeros((128, H + 2), dtype=BF16)
    xin[:, 1 : H + 1] = xt.reshape(128, H).astype(BF16)
    xin[64:128, 0] = xt[0, :, -1].astype(BF16)  # x[H-1] left halo of half 1
    xin[0:64, H + 1] = xt[1, :, 0].astype(BF16)  # x[H] right halo of half 0
    return xin


def kernel(data, ltnt, kernel, Wd, bd):
    from concourse import bass_utils

    nc = _get_nc()

    data = np.asarray(data, dtype=np.float32)
    ltnt = np.asarray(ltnt, dtype=np.float32)
    kf = np.asarray(kernel, dtype=np.float32)
    wdf = np.asarray(Wd, dtype=np.float32)
    bdf = np.asarray(bd, dtype=np.float32)

    in_maps = [
        {"xin": make_xin(data[b]), "par": pack_params(ltnt[b], kf, wdf, bdf)}
        for b in range(B)
    ]

    res = bass_utils.run_bass_kernel_spmd(nc, in_maps, core_ids=list(range(B)))

    out = np.empty((B, L, C), dtype=np.float32)
    even = (np.arange(NGRP) % 2 == 0)[None, :, None]
    for b in range(B):
        yo = np.asarray(res.results[b]["yout"]).astype(np.float32)  # [128, H]
        yr = yo.reshape(2, F, NGRP, 512)  # [rowhalf, f, window, l]
        h0 = np.where(even, yr[0], yr[1])  # odd windows come halves-swapped
        h1 = np.where(even, yr[1], yr[0])
        out[b, :H] = h0.transpose(1, 2, 0).reshape(H, F)
        out[b, H:] = h1.transpose(1, 2, 0).reshape(H, F)
    return out
